# revision 21
# baseline (speedup 1.0000x reference)
"""DeepClusteringLoss Trainium2 kernel.

loss = (||V^T V||_F^2 - 2 ||V^T E||_F^2 + ||E^T E||_F^2) / (B*N)
summed over batch, with E = embeddings.reshape(B, N, D), V =
assignments.reshape(B, N, S), N = F*T.

Sharding: data-parallel over batch; one core per batch element; the host
sums the 8 per-core partials (the scalar "all-reduce") and divides by
B*N.

Per-core pipeline (DMA/HBM-bound: 23.07 MB fp32 input @ ~358 GB/s/core
=> ~64.4 us transfer floor):
- GLOBAL partition map: partition p owns rows [p*1024, (p+1)*1024).
  Chunk c = column c of every partition = 128 rows.
- ALL streaming is HWDGE (SP + ACT rings) in fp32: HWDGE descriptor
  generation is RTL (no Q7 SWDGE boot delay, which cost ~3-6 us of
  16-engine idle at the front), every DMA sprays all 16 SDMA engines
  evenly (the SWDGE baseline left engines 11-15 ~6 us underloaded), and
  since HBM (~358 GB/s) binds before the SBUF AXI fabric (435 GB/s),
  streaming fp32 instead of cast-to-fp16 costs no bandwidth.
- V (2 MB) goes first on the ACT ring into a resident fp32 tile; E
  streams as 19 tapered column-slices (14x64 + 48,32,24,16,8 chunks)
  alternating SP/ACT rings through an 8-deep fp32 ring buffer.
- Interleave copies (DVE for E, ACT for V) cast fp32->fp16 while
  building chunk-PAIR operands [V_2q | E_2q | pad20 | V_2q+1 | E_2q+1]
  (128 x 108 fp16): ONE matmul per two chunks -> 512 PE instruction
  pairs.  Even/odd Grams accumulate at PSUM partition bases 0/64; pad
  and cross-term cells are never read.
- Epilogue dumps the two 44x44 diagonal Gram blocks (SP + ACT rings in
  parallel); the host adds them and reduces to the scalar partial in
  float64 (exact).
"""

import os
from contextlib import ExitStack

import numpy as np

import concourse.bacc as bacc
import concourse.mybir as mybir
import concourse.tile as tile
from concourse.bass_utils import run_bass_kernel_spmd

B, F, T, D, S = 8, 256, 512, 40, 4
N = F * T              # rows per core (131072)
SD = S + D             # 44 combined features
H = 48                 # half-width: V(4) | E(40) | pad(4); 48*2B = 32B-aligned
PW = 2 * H             # paired-chunk width (96)
P = 128                # partitions
U = N // P             # rows per partition in the global map (1024)
N_CORES = 8

MM_DT_NAME = os.environ.get("KERNEL_MM_DT", "float16")
RING = os.environ.get("KERNEL_RING", "alt")   # "alt" | "sp"
EBUFS = int(os.environ.get("KERNEL_EBUFS", "10"))
WBUFS = int(os.environ.get("KERNEL_WBUFS", "8"))

# E slice plan: small HEAD slices so the first ring-FIFO completions
# (and with them the first casts + matmuls) land by ~7 us instead of
# ~16 us; big uniform middle for line-rate DMA; small TAIL taper so the
# last-slice copy+matmul+epilogue dependency chain is short.
SLICES = [16, 16, 32, 32] + [64] * 13 + [32, 16, 16, 8, 8, 8, 8]
assert sum(SLICES) == U
assert all(ub % 2 == 0 for ub in SLICES)

# V pieces, boundaries aligned to slice edges: a tiny leading piece
# covering the head slices (so the first V-copies are not gated by a
# megabyte-scale V transfer), then two big pieces.
VCUTS = [0, 96, 544, U]

_nc_cache = {}


def _build_nc(key):
    (mm_dt_name, ring_mode, ebufs, wbufs) = key
    mm_dt = getattr(mybir.dt, mm_dt_name)
    f32 = mybir.dt.float32

    nc = bacc.Bacc("TRN2", target_bir_lowering=False, debug=False)
    E = nc.dram_tensor("embeddings", (N, D), f32, kind="ExternalInput")
    V = nc.dram_tensor("assignments", (N, S), f32, kind="ExternalInput")
    OUT = nc.dram_tensor("partial", (PW, PW), f32, kind="ExternalOutput")

    # global-map DRAM views: partition p <- rows [p*U, (p+1)*U)
    e_g = E[:, :].rearrange("(p u) d -> p (u d)", p=P)   # [128, U*D]
    v_g = V[:, :].rearrange("(p u) s -> p (u s)", p=P)   # [128, U*S]

    with tile.TileContext(nc) as tc, ExitStack() as ctx:
        res_pool = ctx.enter_context(tc.tile_pool(name="res", bufs=1))
        e_pool = ctx.enter_context(tc.tile_pool(name="e", bufs=ebufs))
        w_pool = ctx.enter_context(tc.tile_pool(name="w", bufs=wbufs))
        psum_pool = ctx.enter_context(tc.tile_pool(name="ps", bufs=1, space="PSUM"))
        g_ps = psum_pool.tile([PW, PW], f32, tag="g")

        # V as three fp32 HWDGE pieces (separate tiles so early slices
        # depend only on the piece that covers them).  The tiny first
        # piece rides at the very head of the SP ring; the big pieces
        # are interleaved into the rings after the head E slices.
        v_tiles = []
        for j in range(len(VCUTS) - 1):
            lo, hi = VCUTS[j], VCUTS[j + 1]
            v_t = res_pool.tile([P, (hi - lo) * S], f32, tag=f"v{j}")
            v_tiles.append((v_t, lo, hi))
        nc.sync.dma_start(
            out=v_tiles[0][0][:], in_=v_g[:, VCUTS[0] * S:VCUTS[1] * S])

        pair = 0
        c0 = 0
        for k, ub in enumerate(SLICES):
            last = k == len(SLICES) - 1
            # E slice: fp32 HWDGE DMA into one of `ebufs` ring slots.
            e_t = e_pool.tile([P, ub * D], f32, tag="e")
            eng = nc.sync if (ring_mode == "sp" or k % 2 == 0) else nc.scalar
            eng.dma_start(out=e_t[:], in_=e_g[:, c0 * D:(c0 + ub) * D])
            if k == 2:
                nc.sync.dma_start(
                    out=v_tiles[1][0][:],
                    in_=v_g[:, VCUTS[1] * S:VCUTS[2] * S])
            elif k == 3:
                nc.scalar.dma_start(
                    out=v_tiles[2][0][:],
                    in_=v_g[:, VCUTS[2] * S:VCUTS[3] * S])

            nq = ub // 2
            w_t = w_pool.tile([P, nq * PW], mm_dt, tag="w")
            # 4D views: one cast per slice fills BOTH halves of every
            # pair.  Copies run on DVE only: the ACT sequencer must stay
            # DMA-only, or its in-order stream blocks tail E DMA issue
            # behind PE-gated copy waits.
            w5 = w_t[:].rearrange("p (q h c) -> p q h c", h=2, c=H)
            e3 = e_t[:].rearrange("p (q h d) -> p q h d", h=2, d=D)
            v_src, vlo, vhi = next(
                vt for vt in v_tiles if vt[1] <= c0 < vt[2])
            assert c0 + ub <= vhi
            vc0 = c0 - vlo
            v3 = v_src[:, vc0 * S:(vc0 + ub) * S].rearrange(
                "p (q h s) -> p q h s", h=2, s=S)
            nc.vector.tensor_copy(w5[:, :, :, S:SD], e3)
            nc.vector.tensor_copy(w5[:, :, :, 0:S], v3)
            for q in range(nq):
                wq = w_t[:, q * PW:(q + 1) * PW]
                nc.tensor.matmul(
                    g_ps[:], wq, wq,
                    start=(pair == 0),
                    stop=(last and q == nq - 1),
                )
                pair += 1
            c0 += ub

        # Epilogue: dump only the two 44x44 diagonal Gram blocks of the
        # PSUM accumulator, each on its own HWDGE ring (SP and ACT) so
        # the descriptor generation for the two OUT transfers runs in
        # parallel; the host adds the blocks and reduces to the scalar
        # partial (exact, in float64) alongside the cross-core sum.
        # Partition-start legality: patterns may start at 0/32/64 and,
        # when starting at 32, cover at most 32 partitions.  The odd
        # Gram block lives at [48:92, 48:92], so dump it as two pieces:
        # rows 48:64 ride a 32-partition access at base 32, rows 64:92
        # a 28-partition access at base 64.
        ep = ctx.enter_context(tc.tile_pool(name="ep", bufs=1))
        ge_sb = ep.tile([SD, SD], f32, tag="ge")
        gl_sb = ep.tile([64, SD], f32, tag="gl")
        gh_sb = ep.tile([92, SD], f32, tag="gh")
        nc.vector.tensor_copy(ge_sb[:], g_ps[0:SD, 0:SD])
        nc.scalar.copy(gl_sb[32:64, :], g_ps[32:64, H:H + SD])
        nc.scalar.copy(gh_sb[64:92, :], g_ps[64:92, H:H + SD])
        nc.sync.dma_start(out=OUT[0:SD, 0:SD], in_=ge_sb[:])
        nc.scalar.dma_start(out=OUT[32:64, H:H + SD], in_=gl_sb[32:64, :])
        nc.scalar.dma_start(out=OUT[64:92, H:H + SD], in_=gh_sb[64:92, :])

    nc.finalize()
    return nc


def _get_nc():
    key = (MM_DT_NAME, RING, EBUFS, WBUFS)
    if key not in _nc_cache:
        _nc_cache[key] = _build_nc(key)
    return _nc_cache[key]


def _run(embeddings: np.ndarray, assignments: np.ndarray, trace: bool = False):
    nc = _get_nc()
    in_maps = []
    for i in range(N_CORES):
        in_maps.append({
            "embeddings": np.ascontiguousarray(
                embeddings[i].reshape(N, D).astype(np.float32, copy=False)),
            "assignments": np.ascontiguousarray(
                assignments[i].reshape(N, S).astype(np.float32, copy=False)),
        })
    try:
        res = run_bass_kernel_spmd(
            nc, in_maps, core_ids=list(range(N_CORES)), trace=trace
        )
    except Exception:
        res = run_bass_kernel_spmd(
            nc, in_maps, core_ids=list(range(N_CORES)), trace=trace
        )
    partials = []
    for r in res.results:
        gp = np.asarray(r["partial"], dtype=np.float64)
        G = gp[0:SD, 0:SD] + gp[H:H + SD, H:H + SD]
        bm = G[0:S, S:SD]
        partials.append(np.sum(G * G) - 4.0 * np.sum(bm * bm))
    total = np.float32(np.sum(np.asarray(partials, dtype=np.float64)) / (B * N))
    return np.asarray(total, dtype=np.float32), res


def kernel(embeddings: np.ndarray, assignments: np.ndarray) -> np.ndarray:
    out, _ = _run(embeddings, assignments, trace=False)
    return out
